# revision 22
# baseline (speedup 1.0000x reference)
"""GatedAttention Trainium2 kernel.

Math (per batch b):
  Qw = x @ Wq + bq            (N, A)
  Kw = x @ Wk + bk            (N, A)
  g  = sigmoid(Qw @ Wv + bv)  (N,)
  S  = Qw @ Kw^T, diag -> -inf
  P  = softmax(S, axis=0)     (column softmax)
  out = (1-g)[:,None] * P + g[:,None] * I

Sharding: 8 cores = 4 batches x 2 column-halves of the score matrix.
Column softmax is independent per column, so no cross-core reduction for
the softmax itself.

Qw partial dedup (hybrid): a column-shard core needs Qw for ALL 4096
rows but its own x covers only 2048. The remote 2048 rows split:
 - quarter [2048:3072) (ch2) arrives from the pair core via ONE
   pair-wise AllGather of its qwt[:, 0:1024] (1MB fp16 DRAM bounce;
   runs on the TOPSP/SDMA collective silicon, doorbell fires at ~38us
   right after the first two Q i-blocks, lands ~77us under the score
   prefix). The AG output is rank-ordered, so the peer slice is read
   back with an indirect row-index DMA whose index vector is a
   per-core host input - the program stays pure SPMD.
 - quarter [3072:4096) (ch3) is projected locally from host-staged
   extra x columns (+64 matmuls) - cheaper than waiting on a second
   serialized collective (pair AGs run ~29GB/s and back-to-back
   collectives don't overlap).
All gates compute locally (z = Qw@Wv once qwt exists), so nothing
gates the collective doorbell except the qwt casts. Total PE drops
944 -> 876 matmuls at ~227ns.

Device layout: scores computed transposed, sT[j, i] tiles (j on
partitions) so the softmax reduction over i is a free-axis reduction.
The i axis is host-permuted so each core's diagonal block sits at
i in [0, 2048).

Dtypes: x / Wq / Wk ship as fp16 (half the HBM read traffic) and are
upcast on-device to fp32r, which streams through the PE at 227ns per
512-row matmul - measurably faster than fp16/bf16 operands (259ns).
SWDGE casting DMAs are ~5x slower than plain ones, so all casts are
explicit DVE ops. Projections/scores accumulate in fp32 PSUM; Exp
output in bf16; the exchange wire format is fp16.

Score schedule: a PREFIX of local-column halves buffers the AG
latency; the drain interleaves untouched tiles (PE-heavy lo+hi matmul
units) with prefix-tile completions (DVE-heavy finalize) so the DVE
normalize stream (~5.6us/tile stt+diag) stays hidden under PE work.
"""
import numpy as np

import concourse.bacc as bacc
import concourse.bass as bass
import concourse.mybir as mybir
import concourse.tile as tile
from concourse.bass_utils import run_bass_kernel_spmd

FP32 = mybir.dt.float32
FP32R = mybir.dt.float32r
FP16 = mybir.dt.float16
BF16 = mybir.dt.bfloat16
I32 = mybir.dt.int32
AF = mybir.ActivationFunctionType
ALU = mybir.AluOpType
AX = mybir.AxisListType

B, N, H, A = 4, 4096, 1024, 512
NSH = N // 2          # per-core column shard / locally projected rows
XW = NSH + 1024       # staged x columns: local 2048 + remote-ch3 1024
NEG = -1.0e30
PREFIX = 8            # score tiles that run local-cols-only up front

_CACHE = {}


def _build():
    nc = bacc.Bacc("TRN2", target_bir_lowering=False, debug=False, num_devices=8)
    xq = nc.dram_tensor("xq", [H, XW], FP16, kind="ExternalInput").ap()
    wq = nc.dram_tensor("wq", [H, A], FP16, kind="ExternalInput").ap()
    wk = nc.dram_tensor("wk", [H, A], FP16, kind="ExternalInput").ap()
    misc = nc.dram_tensor("misc", [128, 18], FP32, kind="ExternalInput").ap()
    eye = nc.dram_tensor("eye", [128, 128], FP32, kind="ExternalInput").ap()
    idx = nc.dram_tensor("idx", [128, 4], I32, kind="ExternalInput").ap()
    out = nc.dram_tensor("out", [NSH, N], BF16, kind="ExternalOutput").ap()

    with tile.TileContext(nc) as tc:
        with (
            tc.tile_pool(name="const", bufs=1) as cpool,
            tc.tile_pool(name="proj_out", bufs=1) as qkpool,
            tc.tile_pool(name="bcast", bufs=1) as bcp,
            tc.tile_pool(name="gaterow", bufs=8) as gtmp,
            tc.tile_pool(name="xchg", bufs=4) as xch,
            tc.tile_pool(name="gateps", bufs=1, space="PSUM") as gps,
            tc.tile_pool(name="dram", bufs=1, space="DRAM") as dram,
        ):
            # ---- memset-only constants first: the warm-up burst depends
            # only on these, so the PE starts right after the preamble.
            ones_f = cpool.tile([1, 128], FP32, tag="onesf", name="onesf")
            nc.vector.memset(ones_f[:], 1.0)
            ones_r = cpool.tile([1, 128], FP32R, tag="ones", name="ones")
            nc.vector.tensor_copy(ones_r[:], ones_f[:])
            ones_h = cpool.tile([1, 128], FP16, tag="onesh", name="onesh")
            nc.vector.tensor_copy(ones_h[:], ones_f[:])

            # ---- DMA'd constants
            ident = cpool.tile([128, 128], FP32, tag="ident", name="ident")
            nc.sync.dma_start(ident[:], eye)
            misc_sb = cpool.tile([128, 18], FP32, tag="misc", name="misc")
            nc.gpsimd.dma_start(misc_sb[:], misc)
            idx_sb = cpool.tile([128, 4], I32, tag="idx", name="idx")
            nc.gpsimd.dma_start(idx_sb[:], idx)
            identb = cpool.tile([128, 128], BF16, tag="identb", name="identb")
            nc.vector.tensor_copy(identb[:], ident[:])
            dneg = cpool.tile([128, 128], FP32, tag="dneg", name="dneg")
            nc.vector.tensor_scalar(dneg[:], ident[:], NEG, None, op0=ALU.mult)
            misc_r = cpool.tile([128, 18], FP32R, tag="miscr", name="miscr")
            nc.vector.tensor_copy(misc_r[:], misc_sb[:])

            # ---- persistent projection outputs (fp32r) ----
            qwt = [qkpool.tile([128, N], FP32R, tag=f"qwt{a}", name=f"qwt{a}")
                   for a in range(4)]
            kwt = [qkpool.tile([128, NSH], FP32R, tag=f"kwt{a}", name=f"kwt{a}")
                   for a in range(4)]
            # gate broadcast planes (bf16): g1m[p,i] = 1-g_i (all i),
            # gbc[p,i] = g_i (local i only - the diagonal never lands in
            # the remote half)
            g1m_bc = bcp.tile([128, N], BF16, tag="g1mbc", name="g1mbc")
            g_bc = bcp.tile([128, NSH], BF16, tag="gbc", name="gbc")

            # AllGather bounce buffers (DRAM): rows = qwt a-index,
            # cols = local i [0:1024)
            agin = dram.tile([512, 1024], FP16, tag="agi", name="agi")
            agout = dram.tile([1024, 1024], FP16, tag="ago", name="ago")

            # 8 x 512-wide projection/gate blocks: 0-3 local, 4-5 = the AG
            # quarter (ch2), 6-7 = locally projected remote ch3
            blk_off = [0, 512, 1024, 1536, 2048, 2560, 3072, 3584]
            grows = [None] * 8

            def emit_gate(blk, g2_too=True):
                # z = Qw @ Wv (dup pair cols so both outputs land on
                # partition 0); 1-g = sigmoid(-z-bv), g = sigmoid(z+bv)
                o = blk_off[blk]
                pzc = gps.tile([2, 512], FP32, tag="zr", name="zr")
                for a in range(4):
                    nc.tensor.matmul(
                        pzc[:], misc_r[:, 8 + 2 * a:10 + 2 * a],
                        qwt[a][:, o:o + 512],
                        start=(a == 0), stop=(a == 3))
                g1 = gtmp.tile([1, 512], FP16, tag="g1", name="g1")
                nc.scalar.activation(g1[:], pzc[0:1, :], AF.Sigmoid,
                                     scale=-1.0, bias=misc_sb[0:1, 17:18])
                g2 = None
                if g2_too:
                    g2 = gtmp.tile([1, 512], FP16, tag="g2", name="g2")
                    nc.scalar.activation(g2[:], pzc[0:1, :], AF.Sigmoid,
                                         bias=misc_sb[0:1, 16:17])
                grows[blk] = (g1, g2)

            def emit_bcast(blk):
                # gate planes for one 512 block via rank-1 ones matmuls
                g1, g2 = grows[blk]
                o = blk_off[blk]
                pb = gps.tile([128, 512], FP32, tag="pb", name="pb")
                nc.tensor.matmul(pb[:], ones_h[:], g1[:], start=True,
                                 stop=True)
                nc.vector.tensor_copy(g1m_bc[:, o:o + 512], pb[:])
                if g2 is not None:
                    pb2 = gps.tile([128, 512], FP32, tag="pb", name="pb")
                    nc.tensor.matmul(pb2[:], ones_h[:], g2[:], start=True,
                                     stop=True)
                    nc.scalar.copy(g_bc[:, o:o + 512], pb2[:])

            # ---- projections + fused gate + exchange ----
            with (
                tc.tile_pool(name="wtiles", bufs=1) as wpool,
                tc.tile_pool(name="wstage", bufs=3) as wst,
                tc.tile_pool(name="xstage", bufs=10) as xst,
                tc.tile_pool(name="xslices", bufs=10) as xpool,
                tc.tile_pool(name="projps", bufs=4, space="PSUM") as ppool,
            ):
                # PE warm-up: keeps the HAM activity monitor busy during the
                # DMA lead-in so the first real matmuls run at full clock.
                warm = ppool.tile([128, 512], FP32, tag="ps", name="warm")
                for _ in range(32):
                    nc.tensor.matmul(warm[0:64, 0:64], ones_r[:, 0:64],
                                     ones_r[:, 0:64], start=True, stop=True)

                def load_w(dram_t, h, lst, tag):
                    wt = wst.tile([128, A], FP16, tag="wst", name="wst")
                    nc.sync.dma_start(wt[:], dram_t[h * 128:(h + 1) * 128, :])
                    wr = wpool.tile([128, A], FP32R, tag=f"{tag}{h}",
                                    name=f"{tag}{h}")
                    nc.vector.tensor_copy(wr[:], wt[:])
                    lst.append(wr)

                wqr, wkr = [], []
                for h in range(8):
                    load_w(wq, h, wqr, "wqr")

                def load_x(xcol):
                    # plain fp16 DMAs on sync (SWDGE casting DMAs are slow),
                    # explicit DVE upcasts to fp32r
                    xs = []
                    for h in range(8):
                        xt = xst.tile([128, 512], FP16, tag="xst", name="xst")
                        nc.sync.dma_start(
                            xt[:], xq[h * 128:(h + 1) * 128, xcol:xcol + 512])
                        xr = xpool.tile([128, 512], FP32R, tag="xr", name="xr")
                        nc.vector.tensor_copy(xr[:], xt[:])
                        xs.append(xr)
                    return xs

                def proj(xs, wlist, dst, dcol, bcol):
                    for a in range(4):
                        pq = ppool.tile([128, 512], FP32, tag="ps", name="ps")
                        for h in range(8):
                            nc.tensor.matmul(pq[:],
                                             wlist[h][:, a * 128:(a + 1) * 128],
                                             xs[h][:], start=(h == 0),
                                             stop=(h == 7))
                        nc.scalar.activation(dst[a][:, dcol:dcol + 512], pq[:],
                                             AF.Identity,
                                             bias=misc_sb[:, bcol + a:bcol + a + 1])

                # ---- Q pass: blocks 0-3 local, 6-7 = remote ch3 (from the
                # extra staged x columns). The exchange fires after block 1.
                qplan = [(0, 0), (1, 512), (2, 1024), (3, 1536),
                         (6, 2048), (7, 2560)]  # (block, x column)
                xs_next = load_x(qplan[0][1])
                for k, (blk, xcol) in enumerate(qplan):
                    xs = xs_next
                    xs_next = (load_x(qplan[k + 1][1])
                               if k + 1 < len(qplan) else None)
                    proj(xs, wqr, qwt, blk_off[blk], 0)
                    if k >= 1:
                        emit_gate(qplan[k - 1][0], g2_too=(k - 1 < 4))
                    if k == 2:
                        # ship qwt[:, 0:1024] through the pair AllGather
                        for a in range(4):
                            xc = xch.tile([128, 1024], FP16, tag="xc",
                                          name="xc")
                            nc.vector.tensor_copy(xc[:], qwt[a][:, 0:1024])
                            nc.gpsimd.dma_start(
                                agin[a * 128:(a + 1) * 128, :], xc[:])
                        nc.gpsimd.collective_compute(
                            "AllGather", ALU.bypass,
                            replica_groups=[[0, 1], [2, 3], [4, 5], [6, 7]],
                            ins=[agin[:].opt()], outs=[agout[:].opt()],
                        )
                    if k == 4:
                        for h in range(8):
                            load_w(wk, h, wkr, "wkr")
                emit_gate(7, g2_too=False)

                # ---- K pass (4 local blocks; x re-read from HBM - the sync
                # queue is quiet here)
                xs_next = load_x(0)
                for ib in range(4):
                    xs = xs_next
                    xs_next = load_x((ib + 1) * 512) if ib < 3 else None
                    proj(xs, wkr, kwt, ib * 512, 4)
                    emit_bcast(ib)                 # blocks 0..3
                    if ib in (1, 2):
                        emit_bcast(5 + ib)         # blocks 6,7

            # ---- AG readback (indirect gathers on gpsimd wait on the
            # collective; emitted after the prefix so their waits never
            # head-block ready PE work), then gates for the AG quarter
            def readback():
                for a in range(4):
                    gt = xch.tile([128, 1024], FP16, tag="gt", name="gt")
                    nc.gpsimd.indirect_dma_start(
                        out=gt[:], out_offset=None,
                        in_=agout[:],
                        in_offset=bass.IndirectOffsetOnAxis(
                            ap=idx_sb[:, a:a + 1], axis=0),
                    )
                    # upcast on the scalar engine: the DVE queue is in-order
                    # and a collective-gated cast there head-blocks the
                    # K-pass x upcasts behind it
                    nc.scalar.copy(qwt[a][:, NSH:NSH + 1024], gt[:])

            # ---- score loop over 16 column tiles (output stays transposed)
            with (
                tc.tile_pool(name="explo", bufs=11) as eplo,
                tc.tile_pool(name="exphi", bufs=3) as ephi,
                tc.tile_pool(name="dsum", bufs=20) as dpool,
                tc.tile_pool(name="diag", bufs=2) as dzpool,
                tc.tile_pool(name="scoreps", bufs=3, space="PSUM") as sps,
            ):
                exp_lo = [None] * 16
                dsums = [None] * 16

                def score_half(t, lo):
                    if lo:
                        et = eplo.tile([128, NSH], BF16, tag="el", name="el")
                        exp_lo[t] = et
                        ds = dpool.tile([128, 4], FP32, tag="ds", name="ds")
                        dsums[t] = ds
                        chs = (0, 1)
                    else:
                        et = ephi.tile([128, NSH], BF16, tag="eh", name="eh")
                        ds = dsums[t]
                        chs = (2, 3)
                    dch = (t * 128) // 1024
                    for ch in chs:
                        ps = sps.tile([128, 1024], FP32, tag="sc", name="sc")
                        for sub in range(2):
                            o = ch * 1024 + sub * 512
                            for a in range(4):
                                nc.tensor.matmul(ps[:, sub * 512:(sub + 1) * 512],
                                                 kwt[a][:, t * 128:(t + 1) * 128],
                                                 qwt[a][:, o:o + 512],
                                                 start=(a == 0), stop=(a == 3))
                        if ch == dch:
                            off = t * 128 - ch * 1024
                            nc.vector.tensor_add(ps[:, off:off + 128],
                                                 ps[:, off:off + 128], dneg[:])
                        co = (ch - chs[0]) * 1024
                        nc.scalar.activation(et[:, co:co + 1024], ps[:],
                                             AF.Exp, accum_out=ds[:, ch:ch + 1])
                    return et

                def finalize(t, ehi, last=False):
                    elo = exp_lo[t]
                    ds = dsums[t]
                    rcol = dpool.tile([128, 1], FP32, tag="r", name="r")
                    nc.vector.tensor_reduce(rcol[:], ds[:], axis=AX.X,
                                            op=ALU.add)
                    nc.vector.reciprocal(rcol[:], rcol[:])
                    eng, eng2 = ((nc.sync, nc.gpsimd) if t % 2 == 1
                                 else (nc.gpsimd, nc.sync))
                    for ch in range(2):
                        sl = slice(ch * 1024, (ch + 1) * 1024)
                        nc.vector.scalar_tensor_tensor(
                            elo[:, sl], elo[:, sl], rcol[:],
                            g1m_bc[:, sl], op0=ALU.mult, op1=ALU.mult)
                    dz = dzpool.tile([128, 128], BF16, tag="dz", name="dz")
                    nc.gpsimd.tensor_mul(dz[:], identb[:],
                                         g_bc[:, t * 128:(t + 1) * 128])
                    nc.gpsimd.tensor_add(elo[:, t * 128:(t + 1) * 128],
                                         elo[:, t * 128:(t + 1) * 128], dz[:])
                    eng.dma_start(out[t * 128:(t + 1) * 128, 0:NSH], elo[:])
                    for ch in range(2):
                        sl = slice(ch * 1024, (ch + 1) * 1024)
                        nc.vector.scalar_tensor_tensor(
                            ehi[:, sl], ehi[:, sl], rcol[:],
                            g1m_bc[:, NSH + ch * 1024:NSH + (ch + 1) * 1024],
                            op0=ALU.mult, op1=ALU.mult)
                        if last:
                            e = eng if ch == 0 else eng2
                            e.dma_start(
                                out[t * 128:(t + 1) * 128,
                                    NSH + ch * 1024:NSH + (ch + 1) * 1024],
                                ehi[:, sl])
                    if not last:
                        eng2.dma_start(out[t * 128:(t + 1) * 128, NSH:N],
                                       ehi[:])

                # prefix: local-column halves (AG latency buffer)
                for t in range(PREFIX):
                    score_half(t, lo=True)
                # AG quarter: readback, cast, its gates + planes
                readback()
                emit_gate(4, g2_too=False)
                emit_gate(5, g2_too=False)
                emit_bcast(4)
                emit_bcast(5)
                # drain: interleave untouched tiles (PE-heavy) with prefix
                # completions (DVE-heavy finalize) to keep both engines fed
                plan = []
                for k in range(16 - PREFIX):
                    plan.append(PREFIX + k)
                    plan.append(k)
                plan += list(range(16 - PREFIX, PREFIX))
                for n_, t in enumerate(plan):
                    if t >= PREFIX:
                        score_half(t, lo=True)
                    ehi = score_half(t, lo=False)
                    finalize(t, ehi, last=(n_ == len(plan) - 1))
    nc.compile()
    return nc


def kernel(x, Wq, bq, Wk, bk, Wv, bv, _trace=False, _tmpdir=None):
    x = np.asarray(x, dtype=np.float32)
    if "nc" not in _CACHE:
        _CACHE["nc"] = _build()
    nc = _CACHE["nc"]

    bv_f = np.float32(np.asarray(bv).reshape(())[()])
    eye_np = np.eye(128, dtype=np.float32)
    misc = np.zeros((128, 18), dtype=np.float32)
    misc[:, 0:4] = np.asarray(bq, np.float32).reshape(4, 128).T
    misc[:, 4:8] = np.asarray(bk, np.float32).reshape(4, 128).T
    wv_c = np.asarray(Wv, np.float32).reshape(4, 128).T
    misc[:, 8:16:2] = wv_c
    misc[:, 9:16:2] = wv_c
    misc[:, 16] = bv_f
    misc[:, 17] = -bv_f
    wq_np = np.ascontiguousarray(np.asarray(Wq, np.float32).astype(np.float16))
    wk_np = np.ascontiguousarray(np.asarray(Wk, np.float32).astype(np.float16))

    in_maps = []
    for c in range(8):
        b, h = c // 2, c % 2
        xT = x[b].T.astype(np.float16)  # (H, N)
        # local rows [h*2048,(h+1)*2048) + remote-ch3 rows (the second
        # half of the peer's range, = perm-i [3072:4096))
        rem3 = slice((1 - h) * NSH + 1024, (1 - h) * NSH + 2048)
        xqc = np.ascontiguousarray(np.concatenate(
            [xT[:, h * NSH:(h + 1) * NSH], xT[:, rem3]], axis=1))
        idx = np.zeros((128, 4), dtype=np.int32)
        base = (1 - h) * 512
        for a in range(4):
            idx[:, a] = base + a * 128 + np.arange(128)
        in_maps.append({"xq": xqc, "wq": wq_np, "wk": wk_np, "misc": misc,
                        "eye": eye_np, "idx": idx})

    res = run_bass_kernel_spmd(nc, in_maps, list(range(8)), trace=_trace,
                               tmpdir=_tmpdir)

    outp = np.empty((B, N, N), dtype=np.float32)
    for c in range(8):
        b, h = c // 2, c % 2
        O = np.asarray(res.results[c]["out"]).astype(np.float32).T  # (i_perm, j)
        js = slice(h * NSH, (h + 1) * NSH)
        outp[b, h * NSH:(h + 1) * NSH, js] = O[:NSH]
        outp[b, (1 - h) * NSH:(2 - h) * NSH, js] = O[NSH:]
    if _trace:
        return outp, res
    return outp


# revision 23
# speedup vs baseline: 1.0333x; 1.0333x over previous
"""GatedAttention Trainium2 kernel.

Math (per batch b):
  Qw = x @ Wq + bq            (N, A)
  Kw = x @ Wk + bk            (N, A)
  g  = sigmoid(Qw @ Wv + bv)  (N,)
  S  = Qw @ Kw^T, diag -> -inf
  P  = softmax(S, axis=0)     (column softmax)
  out = (1-g)[:,None] * P + g[:,None] * I

Sharding: 8 cores = 4 batches x 2 column-halves of the score matrix.
Column softmax is independent per column, so no cross-core reduction for
the softmax itself.

Qw partial dedup (hybrid): a column-shard core needs Qw for ALL 4096
rows but its own x covers only 2048. The remote 2048 rows split:
 - quarter [2048:3072) (ch2) arrives from the pair core via ONE
   pair-wise AllGather of its qwt[:, 0:1024] (1MB fp16 DRAM bounce;
   runs on the TOPSP/SDMA collective silicon, doorbell fires at ~38us
   right after the first two Q i-blocks, lands ~77us under the score
   prefix). The AG output is rank-ordered, so the peer slice is read
   back with an indirect row-index DMA whose index vector is a
   per-core host input - the program stays pure SPMD.
 - quarter [3072:4096) (ch3) is projected locally from host-staged
   extra x columns (+64 matmuls) - cheaper than waiting on a second
   serialized collective (pair AGs run ~29GB/s and back-to-back
   collectives don't overlap).
All gates compute locally (z = Qw@Wv once qwt exists), so nothing
gates the collective doorbell except the qwt casts. Total PE drops
944 -> 876 matmuls at ~227ns.

Device layout: scores computed transposed, sT[j, i] tiles (j on
partitions) so the softmax reduction over i is a free-axis reduction.
The i axis is host-permuted so each core's diagonal block sits at
i in [0, 2048).

Dtypes: x / Wq / Wk ship as fp16 (half the HBM read traffic) and are
upcast on-device to fp32r, which streams through the PE at 227ns per
512-row matmul - measurably faster than fp16/bf16 operands (259ns).
SWDGE casting DMAs are ~5x slower than plain ones, so all casts are
explicit DVE ops. Projections/scores accumulate in fp32 PSUM; Exp
output in bf16; the exchange wire format is fp16.

Score schedule: a PREFIX of local-column halves buffers the AG
latency; the drain interleaves untouched tiles (PE-heavy lo+hi matmul
units) with prefix-tile completions (DVE-heavy finalize) so the DVE
normalize stream (~5.6us/tile stt+diag) stays hidden under PE work.
"""
import numpy as np

import concourse.bacc as bacc
import concourse.bass as bass
import concourse.mybir as mybir
import concourse.tile as tile
from concourse.bass_utils import run_bass_kernel_spmd

FP32 = mybir.dt.float32
FP32R = mybir.dt.float32r
FP16 = mybir.dt.float16
BF16 = mybir.dt.bfloat16
I32 = mybir.dt.int32
AF = mybir.ActivationFunctionType
ALU = mybir.AluOpType
AX = mybir.AxisListType

B, N, H, A = 4, 4096, 1024, 512
NSH = N // 2          # per-core column shard / locally projected rows
XW = NSH + 1024       # staged x columns: local 2048 + remote-ch3 1024
NEG = -1.0e30
PREFIX = 8            # score tiles that run local-cols-only up front

_CACHE = {}


def _build():
    nc = bacc.Bacc("TRN2", target_bir_lowering=False, debug=False, num_devices=8)
    xq = nc.dram_tensor("xq", [H, XW], FP16, kind="ExternalInput").ap()
    wq = nc.dram_tensor("wq", [H, A], FP16, kind="ExternalInput").ap()
    wk = nc.dram_tensor("wk", [H, A], FP16, kind="ExternalInput").ap()
    misc = nc.dram_tensor("misc", [128, 18], FP32, kind="ExternalInput").ap()
    eye = nc.dram_tensor("eye", [128, 128], FP32, kind="ExternalInput").ap()
    idx = nc.dram_tensor("idx", [128, 4], I32, kind="ExternalInput").ap()
    out = nc.dram_tensor("out", [NSH, N], BF16, kind="ExternalOutput").ap()

    with tile.TileContext(nc) as tc:
        with (
            tc.tile_pool(name="const", bufs=1) as cpool,
            tc.tile_pool(name="proj_out", bufs=1) as qkpool,
            tc.tile_pool(name="bcast", bufs=1) as bcp,
            tc.tile_pool(name="gaterow", bufs=8) as gtmp,
            tc.tile_pool(name="xchg", bufs=4) as xch,
            tc.tile_pool(name="gateps", bufs=1, space="PSUM") as gps,
            tc.tile_pool(name="dram", bufs=1, space="DRAM") as dram,
        ):
            # ---- memset-only constants first: the warm-up burst depends
            # only on these, so the PE starts right after the preamble.
            ones_f = cpool.tile([1, 128], FP32, tag="onesf", name="onesf")
            nc.vector.memset(ones_f[:], 1.0)
            ones_r = cpool.tile([1, 128], FP32R, tag="ones", name="ones")
            nc.vector.tensor_copy(ones_r[:], ones_f[:])
            ones_h = cpool.tile([1, 128], FP16, tag="onesh", name="onesh")
            nc.vector.tensor_copy(ones_h[:], ones_f[:])

            # ---- DMA'd constants
            ident = cpool.tile([128, 128], FP32, tag="ident", name="ident")
            nc.sync.dma_start(ident[:], eye)
            misc_sb = cpool.tile([128, 18], FP32, tag="misc", name="misc")
            nc.gpsimd.dma_start(misc_sb[:], misc)
            idx_sb = cpool.tile([128, 4], I32, tag="idx", name="idx")
            nc.gpsimd.dma_start(idx_sb[:], idx)
            identb = cpool.tile([128, 128], BF16, tag="identb", name="identb")
            nc.vector.tensor_copy(identb[:], ident[:])
            dneg = cpool.tile([128, 128], FP32, tag="dneg", name="dneg")
            nc.vector.tensor_scalar(dneg[:], ident[:], NEG, None, op0=ALU.mult)
            misc_r = cpool.tile([128, 18], FP32R, tag="miscr", name="miscr")
            nc.vector.tensor_copy(misc_r[:], misc_sb[:])

            # ---- persistent projection outputs (fp32r) ----
            qwt = [qkpool.tile([128, N], FP32R, tag=f"qwt{a}", name=f"qwt{a}")
                   for a in range(4)]
            kwt = [qkpool.tile([128, NSH], FP32R, tag=f"kwt{a}", name=f"kwt{a}")
                   for a in range(4)]
            # gate broadcast planes (bf16): g1m[p,i] = 1-g_i (all i),
            # gbc[p,i] = g_i (local i only - the diagonal never lands in
            # the remote half)
            g1m_bc = bcp.tile([128, N], BF16, tag="g1mbc", name="g1mbc")
            g_bc = bcp.tile([128, NSH], BF16, tag="gbc", name="gbc")

            # AllGather bounce buffers (DRAM): rows = qwt a-index,
            # cols = local i [0:1024)
            agin = dram.tile([512, 1024], FP16, tag="agi", name="agi")
            agout = dram.tile([1024, 1024], FP16, tag="ago", name="ago")

            # 8 x 512-wide projection/gate blocks: 0-3 local, 4-5 = the AG
            # quarter (ch2), 6-7 = locally projected remote ch3
            blk_off = [0, 512, 1024, 1536, 2048, 2560, 3072, 3584]
            grows = [None] * 8

            def emit_gate(blk, g2_too=True):
                # z = Qw @ Wv (dup pair cols so both outputs land on
                # partition 0); 1-g = sigmoid(-z-bv), g = sigmoid(z+bv)
                o = blk_off[blk]
                pzc = gps.tile([2, 512], FP32, tag="zr", name="zr")
                for a in range(4):
                    nc.tensor.matmul(
                        pzc[:], misc_r[:, 8 + 2 * a:10 + 2 * a],
                        qwt[a][:, o:o + 512],
                        start=(a == 0), stop=(a == 3))
                g1 = gtmp.tile([1, 512], FP16, tag="g1", name="g1")
                nc.scalar.activation(g1[:], pzc[0:1, :], AF.Sigmoid,
                                     scale=-1.0, bias=misc_sb[0:1, 17:18])
                g2 = None
                if g2_too:
                    g2 = gtmp.tile([1, 512], FP16, tag="g2", name="g2")
                    nc.scalar.activation(g2[:], pzc[0:1, :], AF.Sigmoid,
                                         bias=misc_sb[0:1, 16:17])
                grows[blk] = (g1, g2)

            def emit_bcast(blk):
                # gate planes for one 512 block via rank-1 ones matmuls
                g1, g2 = grows[blk]
                o = blk_off[blk]
                pb = gps.tile([128, 512], FP32, tag="pb", name="pb")
                nc.tensor.matmul(pb[:], ones_h[:], g1[:], start=True,
                                 stop=True)
                nc.vector.tensor_copy(g1m_bc[:, o:o + 512], pb[:])
                if g2 is not None:
                    pb2 = gps.tile([128, 512], FP32, tag="pb", name="pb")
                    nc.tensor.matmul(pb2[:], ones_h[:], g2[:], start=True,
                                     stop=True)
                    nc.scalar.copy(g_bc[:, o:o + 512], pb2[:])

            # ---- projections + fused gate + exchange ----
            with (
                tc.tile_pool(name="wtiles", bufs=1) as wpool,
                tc.tile_pool(name="wstage", bufs=3) as wst,
                tc.tile_pool(name="xstage", bufs=10) as xst,
                tc.tile_pool(name="xslices", bufs=10) as xpool,
                tc.tile_pool(name="projps", bufs=4, space="PSUM") as ppool,
            ):
                # PE warm-up: keeps the HAM activity monitor busy during the
                # DMA lead-in so the first real matmuls run at full clock.
                warm = ppool.tile([128, 512], FP32, tag="ps", name="warm")
                for _ in range(32):
                    nc.tensor.matmul(warm[0:64, 0:64], ones_r[:, 0:64],
                                     ones_r[:, 0:64], start=True, stop=True)

                def load_w(dram_t, h, lst, tag):
                    wt = wst.tile([128, A], FP16, tag="wst", name="wst")
                    nc.sync.dma_start(wt[:], dram_t[h * 128:(h + 1) * 128, :])
                    wr = wpool.tile([128, A], FP32R, tag=f"{tag}{h}",
                                    name=f"{tag}{h}")
                    nc.vector.tensor_copy(wr[:], wt[:])
                    lst.append(wr)

                wqr, wkr = [], []
                for h in range(8):
                    load_w(wq, h, wqr, "wqr")

                def load_x(xcol):
                    # plain fp16 DMAs on sync (SWDGE casting DMAs are slow),
                    # explicit DVE upcasts to fp32r
                    xs = []
                    for h in range(8):
                        xt = xst.tile([128, 512], FP16, tag="xst", name="xst")
                        nc.sync.dma_start(
                            xt[:], xq[h * 128:(h + 1) * 128, xcol:xcol + 512])
                        xr = xpool.tile([128, 512], FP32R, tag="xr", name="xr")
                        nc.vector.tensor_copy(xr[:], xt[:])
                        xs.append(xr)
                    return xs

                def proj(xs, wlist, dst, dcol, bcol):
                    for a in range(4):
                        pq = ppool.tile([128, 512], FP32, tag="ps", name="ps")
                        for h in range(8):
                            nc.tensor.matmul(pq[:],
                                             wlist[h][:, a * 128:(a + 1) * 128],
                                             xs[h][:], start=(h == 0),
                                             stop=(h == 7))
                        nc.scalar.activation(dst[a][:, dcol:dcol + 512], pq[:],
                                             AF.Identity,
                                             bias=misc_sb[:, bcol + a:bcol + a + 1])

                # ---- Q pass: blocks 0-3 local, 6-7 = remote ch3 (from the
                # extra staged x columns). The exchange fires after block 1.
                qplan = [(0, 0), (1, 512), (2, 1024), (3, 1536),
                         (6, 2048), (7, 2560)]  # (block, x column)
                xs_next = load_x(qplan[0][1])
                for k, (blk, xcol) in enumerate(qplan):
                    xs = xs_next
                    xs_next = (load_x(qplan[k + 1][1])
                               if k + 1 < len(qplan) else None)
                    proj(xs, wqr, qwt, blk_off[blk], 0)
                    if k >= 1:
                        emit_gate(qplan[k - 1][0], g2_too=(k - 1 < 4))
                    if k == 2:
                        # ship qwt[:, 0:1024] through the pair AllGather
                        for a in range(4):
                            xc = xch.tile([128, 1024], FP16, tag="xc",
                                          name="xc")
                            nc.vector.tensor_copy(xc[:], qwt[a][:, 0:1024])
                            nc.gpsimd.dma_start(
                                agin[a * 128:(a + 1) * 128, :], xc[:])
                        nc.gpsimd.collective_compute(
                            "AllGather", ALU.bypass,
                            replica_groups=[[0, 1], [2, 3], [4, 5], [6, 7]],
                            ins=[agin[:].opt()], outs=[agout[:].opt()],
                        )
                    if k == 4:
                        for h in range(8):
                            load_w(wk, h, wkr, "wkr")
                emit_gate(7, g2_too=False)

                # ---- K pass (4 local blocks; x re-read from HBM - the sync
                # queue is quiet here)
                xs_next = load_x(0)
                for ib in range(4):
                    xs = xs_next
                    xs_next = load_x((ib + 1) * 512) if ib < 3 else None
                    proj(xs, wkr, kwt, ib * 512, 4)
                    emit_bcast(ib)                 # blocks 0..3
                    if ib in (1, 2):
                        emit_bcast(5 + ib)         # blocks 6,7

            # ---- AG readback (indirect gathers on gpsimd wait on the
            # collective; emitted after the prefix so their waits never
            # head-block ready PE work), then gates for the AG quarter
            def readback():
                for a in range(4):
                    gt = xch.tile([128, 1024], FP16, tag="gt", name="gt")
                    nc.gpsimd.indirect_dma_start(
                        out=gt[:], out_offset=None,
                        in_=agout[:],
                        in_offset=bass.IndirectOffsetOnAxis(
                            ap=idx_sb[:, a:a + 1], axis=0),
                    )
                    # upcast off the DVE queue: an in-order collective-gated
                    # cast there would head-block the K-pass x upcasts
                    nc.scalar.copy(qwt[a][:, NSH:NSH + 1024], gt[:])

            # ---- score loop over 16 column tiles (output stays transposed)
            with (
                tc.tile_pool(name="explo", bufs=11) as eplo,
                tc.tile_pool(name="exphi", bufs=3) as ephi,
                tc.tile_pool(name="dsum", bufs=20) as dpool,
                tc.tile_pool(name="diag", bufs=2) as dzpool,
                tc.tile_pool(name="scoreps", bufs=3, space="PSUM") as sps,
            ):
                exp_lo = [None] * 16
                dsums = [None] * 16

                def score_half(t, lo):
                    if lo:
                        et = eplo.tile([128, NSH], BF16, tag="el", name="el")
                        exp_lo[t] = et
                        ds = dpool.tile([128, 4], FP32, tag="ds", name="ds")
                        dsums[t] = ds
                        chs = (0, 1)
                    else:
                        et = ephi.tile([128, NSH], BF16, tag="eh", name="eh")
                        ds = dsums[t]
                        chs = (2, 3)
                    dch = (t * 128) // 1024
                    for ch in chs:
                        ps = sps.tile([128, 1024], FP32, tag="sc", name="sc")
                        for sub in range(2):
                            o = ch * 1024 + sub * 512
                            for a in range(4):
                                nc.tensor.matmul(ps[:, sub * 512:(sub + 1) * 512],
                                                 kwt[a][:, t * 128:(t + 1) * 128],
                                                 qwt[a][:, o:o + 512],
                                                 start=(a == 0), stop=(a == 3))
                        if ch == dch:
                            off = t * 128 - ch * 1024
                            nc.vector.tensor_add(ps[:, off:off + 128],
                                                 ps[:, off:off + 128], dneg[:])
                        co = (ch - chs[0]) * 1024
                        nc.scalar.activation(et[:, co:co + 1024], ps[:],
                                             AF.Exp, accum_out=ds[:, ch:ch + 1])
                    return et

                def finalize(t, ehi, last=False):
                    elo = exp_lo[t]
                    ds = dsums[t]
                    rcol = dpool.tile([128, 1], FP32, tag="r", name="r")
                    nc.vector.tensor_reduce(rcol[:], ds[:], axis=AX.X,
                                            op=ALU.add)
                    nc.vector.reciprocal(rcol[:], rcol[:])
                    eng, eng2 = ((nc.sync, nc.gpsimd) if t % 2 == 1
                                 else (nc.gpsimd, nc.sync))
                    for ch in range(2):
                        sl = slice(ch * 1024, (ch + 1) * 1024)
                        nc.vector.scalar_tensor_tensor(
                            elo[:, sl], elo[:, sl], rcol[:],
                            g1m_bc[:, sl], op0=ALU.mult, op1=ALU.mult)
                    dz = dzpool.tile([128, 128], BF16, tag="dz", name="dz")
                    nc.vector.tensor_mul(dz[:], identb[:],
                                         g_bc[:, t * 128:(t + 1) * 128])
                    nc.vector.tensor_add(elo[:, t * 128:(t + 1) * 128],
                                         elo[:, t * 128:(t + 1) * 128], dz[:])
                    eng.dma_start(out[t * 128:(t + 1) * 128, 0:NSH], elo[:])
                    for ch in range(2):
                        sl = slice(ch * 1024, (ch + 1) * 1024)
                        nc.vector.scalar_tensor_tensor(
                            ehi[:, sl], ehi[:, sl], rcol[:],
                            g1m_bc[:, NSH + ch * 1024:NSH + (ch + 1) * 1024],
                            op0=ALU.mult, op1=ALU.mult)
                        if last:
                            e = eng if ch == 0 else eng2
                            e.dma_start(
                                out[t * 128:(t + 1) * 128,
                                    NSH + ch * 1024:NSH + (ch + 1) * 1024],
                                ehi[:, sl])
                    if not last:
                        eng2.dma_start(out[t * 128:(t + 1) * 128, NSH:N],
                                       ehi[:])

                # AG quarter: readback, cast, its gates + planes
                readback()
                emit_gate(4, g2_too=False)
                emit_gate(5, g2_too=False)
                emit_bcast(4)
                emit_bcast(5)
                # prefix: local-column halves (AG latency buffer)
                for t in range(PREFIX):
                    score_half(t, lo=True)
                # drain: interleave untouched tiles (PE-heavy) with prefix
                # completions (DVE-heavy finalize) to keep both engines fed
                plan = []
                for k in range(16 - PREFIX):
                    plan.append(PREFIX + k)
                    plan.append(k)
                plan += list(range(16 - PREFIX, PREFIX))
                for n_, t in enumerate(plan):
                    if t >= PREFIX:
                        score_half(t, lo=True)
                    ehi = score_half(t, lo=False)
                    finalize(t, ehi, last=(n_ == len(plan) - 1))
    nc.compile()
    return nc


def kernel(x, Wq, bq, Wk, bk, Wv, bv, _trace=False, _tmpdir=None):
    x = np.asarray(x, dtype=np.float32)
    if "nc" not in _CACHE:
        _CACHE["nc"] = _build()
    nc = _CACHE["nc"]

    bv_f = np.float32(np.asarray(bv).reshape(())[()])
    eye_np = np.eye(128, dtype=np.float32)
    misc = np.zeros((128, 18), dtype=np.float32)
    misc[:, 0:4] = np.asarray(bq, np.float32).reshape(4, 128).T
    misc[:, 4:8] = np.asarray(bk, np.float32).reshape(4, 128).T
    wv_c = np.asarray(Wv, np.float32).reshape(4, 128).T
    misc[:, 8:16:2] = wv_c
    misc[:, 9:16:2] = wv_c
    misc[:, 16] = bv_f
    misc[:, 17] = -bv_f
    wq_np = np.ascontiguousarray(np.asarray(Wq, np.float32).astype(np.float16))
    wk_np = np.ascontiguousarray(np.asarray(Wk, np.float32).astype(np.float16))

    in_maps = []
    for c in range(8):
        b, h = c // 2, c % 2
        xT = x[b].T.astype(np.float16)  # (H, N)
        # local rows [h*2048,(h+1)*2048) + remote-ch3 rows (the second
        # half of the peer's range, = perm-i [3072:4096))
        rem3 = slice((1 - h) * NSH + 1024, (1 - h) * NSH + 2048)
        xqc = np.ascontiguousarray(np.concatenate(
            [xT[:, h * NSH:(h + 1) * NSH], xT[:, rem3]], axis=1))
        idx = np.zeros((128, 4), dtype=np.int32)
        base = (1 - h) * 512
        for a in range(4):
            idx[:, a] = base + a * 128 + np.arange(128)
        in_maps.append({"xq": xqc, "wq": wq_np, "wk": wk_np, "misc": misc,
                        "eye": eye_np, "idx": idx})

    res = run_bass_kernel_spmd(nc, in_maps, list(range(8)), trace=_trace,
                               tmpdir=_tmpdir)

    outp = np.empty((B, N, N), dtype=np.float32)
    for c in range(8):
        b, h = c // 2, c % 2
        O = np.asarray(res.results[c]["out"]).astype(np.float32).T  # (i_perm, j)
        js = slice(h * NSH, (h + 1) * NSH)
        outp[b, h * NSH:(h + 1) * NSH, js] = O[:NSH]
        outp[b, (1 - h) * NSH:(2 - h) * NSH, js] = O[NSH:]
    if _trace:
        return outp, res
    return outp


# revision 24
# speedup vs baseline: 1.0376x; 1.0041x over previous
"""GatedAttention Trainium2 kernel.

Math (per batch b):
  Qw = x @ Wq + bq            (N, A)
  Kw = x @ Wk + bk            (N, A)
  g  = sigmoid(Qw @ Wv + bv)  (N,)
  S  = Qw @ Kw^T, diag -> -inf
  P  = softmax(S, axis=0)     (column softmax)
  out = (1-g)[:,None] * P + g[:,None] * I

Sharding: 8 cores = 4 batches x 2 column-halves of the score matrix.
Column softmax is independent per column, so no cross-core reduction for
the softmax itself.

Qw dedup: a column-shard core needs Qw for ALL 4096 rows, but each core
only PROJECTS its own 2048 rows (the ones whose x it loads); the other
half arrives from the pair core (same batch, other column half) via a
pair-wise AllGather (DRAM bounce; runs on the TOPSP/SDMA collective
silicon and overlaps compute). The AG output is rank-ordered, so each
core reads the peer slice back with an indirect (row-index) DMA whose
index vector is a per-core host input - the program stays pure SPMD.
The exchange also carries the peer's gate rows, so gate z-matmuls are
local-only. This cuts projection PE work by a third (944 -> 812
matmuls at ~227ns each).

Device layout: scores computed transposed, sT[j, i] tiles (j on
partitions) so the softmax reduction over i is a free-axis reduction.
The i axis is host-permuted so each core's diagonal block sits at
i in [0, 2048).

Dtypes: x / Wq / Wk ship as fp16 (half the HBM read traffic) and are
upcast on-device to fp32r, which streams through the PE at 227ns per
512-row matmul - measurably faster than fp16/bf16 operands (259ns).
SWDGE casting DMAs are ~5x slower than plain ones, so all casts are
explicit DVE ops. Projections/scores accumulate in fp32 PSUM; Exp
output in bf16; the exchange wire format is fp16.

Schedule: Q-projection pass over the 4 local i-blocks first (x fp16
tiles stay staged in SBUF and are re-cast for the K pass), so the two
1MB AllGathers (qwt cols [0:1024] / [1024:2048] + gate rows) fire at
~35us and ~47us and land before the score loop needs the remote half.
The score loop runs a PREFIX of local-column-only tiles as a latency
buffer, then processes tiles to completion; the DVE finalize stream
(~5.6us/tile: reciprocal-scale x (1-g) stt + diag) stays hidden under
the PE-paced score matmuls.
"""
import numpy as np

import concourse.bacc as bacc
import concourse.bass as bass
import concourse.mybir as mybir
import concourse.tile as tile
from concourse.bass_utils import run_bass_kernel_spmd

FP32 = mybir.dt.float32
FP32R = mybir.dt.float32r
FP16 = mybir.dt.float16
BF16 = mybir.dt.bfloat16
I32 = mybir.dt.int32
AF = mybir.ActivationFunctionType
ALU = mybir.AluOpType
AX = mybir.AxisListType

B, N, H, A = 4, 4096, 1024, 512
NSH = N // 2          # per-core column shard / locally projected rows
NEG = -1.0e30
AGR = 516             # AllGather bounce rows: 512 qwt + g1 + g2 (+pad)
PREFIX = 8            # score tiles that run local-cols-only up front

_CACHE = {}


def _build():
    nc = bacc.Bacc("TRN2", target_bir_lowering=False, debug=False, num_devices=8)
    xq = nc.dram_tensor("xq", [H, NSH], FP16, kind="ExternalInput").ap()
    wq = nc.dram_tensor("wq", [H, A], FP16, kind="ExternalInput").ap()
    wk = nc.dram_tensor("wk", [H, A], FP16, kind="ExternalInput").ap()
    misc = nc.dram_tensor("misc", [128, 18], FP32, kind="ExternalInput").ap()
    eye = nc.dram_tensor("eye", [128, 128], FP32, kind="ExternalInput").ap()
    idx = nc.dram_tensor("idx", [128, 5], I32, kind="ExternalInput").ap()
    out = nc.dram_tensor("out", [NSH, N], BF16, kind="ExternalOutput").ap()

    with tile.TileContext(nc) as tc:
        with (
            tc.tile_pool(name="const", bufs=1) as cpool,
            tc.tile_pool(name="proj_out", bufs=1) as qkpool,
            tc.tile_pool(name="bcast", bufs=1) as bcp,
            tc.tile_pool(name="gaterow", bufs=4) as gtmp,
            tc.tile_pool(name="xchg", bufs=4) as xch,
            tc.tile_pool(name="grpool", bufs=2) as grp,
            tc.tile_pool(name="dram", bufs=1, space="DRAM") as dram,
        ):
            # ---- memset-only constants first: the warm-up burst depends
            # only on these, so the PE starts right after the preamble.
            ones_f = cpool.tile([1, 128], FP32, tag="onesf", name="onesf")
            nc.vector.memset(ones_f[:], 1.0)
            ones_r = cpool.tile([1, 128], FP32R, tag="ones", name="ones")
            nc.vector.tensor_copy(ones_r[:], ones_f[:])
            ones_h = cpool.tile([1, 128], FP16, tag="onesh", name="onesh")
            nc.vector.tensor_copy(ones_h[:], ones_f[:])

            # ---- DMA'd constants
            ident = cpool.tile([128, 128], FP32, tag="ident", name="ident")
            nc.sync.dma_start(ident[:], eye)
            misc_sb = cpool.tile([128, 18], FP32, tag="misc", name="misc")
            nc.gpsimd.dma_start(misc_sb[:], misc)
            idx_sb = cpool.tile([128, 5], I32, tag="idx", name="idx")
            nc.gpsimd.dma_start(idx_sb[:], idx)
            identb = cpool.tile([128, 128], BF16, tag="identb", name="identb")
            nc.vector.tensor_copy(identb[:], ident[:])
            dneg = cpool.tile([128, 128], FP32, tag="dneg", name="dneg")
            nc.vector.tensor_scalar(dneg[:], ident[:], NEG, None, op0=ALU.mult)
            misc_r = cpool.tile([128, 18], FP32R, tag="miscr", name="miscr")
            nc.vector.tensor_copy(misc_r[:], misc_sb[:])

            # ---- persistent projection outputs (fp32r) ----
            qwt = [qkpool.tile([128, N], FP32R, tag=f"qwt{a}", name=f"qwt{a}")
                   for a in range(4)]
            kwt = [qkpool.tile([128, NSH], FP32R, tag=f"kwt{a}", name=f"kwt{a}")
                   for a in range(4)]
            # gate broadcast planes (bf16): g1m[p,i] = 1-g_i (all i),
            # gbc[p,i] = g_i (local i only - the diagonal never lands in the
            # remote half)
            g1m_bc = bcp.tile([128, N], BF16, tag="g1mbc", name="g1mbc")
            g_bc = bcp.tile([128, NSH], BF16, tag="gbc", name="gbc")

            # AllGather bounce buffers (DRAM). agin rows: 0-511 = qwt local
            # (row a), 512 = g1 row, 513 = g2 row; cols = one 1024-wide
            # half of the local i range.
            agin = [dram.tile([AGR, 1024], FP16, tag=f"agi{i}", name=f"agi{i}")
                    for i in range(2)]
            agout = [dram.tile([2 * AGR, 1024], FP16, tag=f"ago{i}",
                               name=f"ago{i}") for i in range(2)]

            # ---- projections + fused gate + exchange ----
            with (
                tc.tile_pool(name="wtiles", bufs=1) as wpool,
                tc.tile_pool(name="wstage", bufs=3) as wst,
                tc.tile_pool(name="xstage", bufs=10) as xst,
                tc.tile_pool(name="xslices", bufs=10) as xpool,
                tc.tile_pool(name="projps", bufs=4, space="PSUM") as ppool,
                tc.tile_pool(name="zrowps", bufs=2, space="PSUM") as zpool,
                tc.tile_pool(name="bcps", bufs=2, space="PSUM") as bps,
            ):
                # PE warm-up: keeps the HAM activity monitor busy during the
                # DMA lead-in so the first real matmuls run at full clock.
                warm = ppool.tile([128, 512], FP32, tag="ps", name="warm")
                for _ in range(32):
                    nc.tensor.matmul(warm[0:64, 0:64], ones_r[:, 0:64],
                                     ones_r[:, 0:64], start=True, stop=True)

                def load_w(dram_t, h, lst, tag):
                    wt = wst.tile([128, A], FP16, tag="wst", name="wst")
                    nc.sync.dma_start(wt[:], dram_t[h * 128:(h + 1) * 128, :])
                    wr = wpool.tile([128, A], FP32R, tag=f"{tag}{h}",
                                    name=f"{tag}{h}")
                    nc.vector.tensor_copy(wr[:], wt[:])
                    lst.append(wr)

                wqr, wkr = [], []
                for h in range(8):
                    load_w(wq, h, wqr, "wqr")

                def load_x(ib):
                    # plain fp16 DMAs on sync (SWDGE casting DMAs are slow),
                    # explicit DVE upcasts to fp32r
                    xs = []
                    for h in range(8):
                        xt = xst.tile([128, 512], FP16, tag="xst", name="xst")
                        nc.sync.dma_start(
                            xt[:], xq[h * 128:(h + 1) * 128,
                                      ib * 512:(ib + 1) * 512])
                        xr = xpool.tile([128, 512], FP32R, tag="xr", name="xr")
                        nc.vector.tensor_copy(xr[:], xt[:])
                        xs.append(xr)
                    return xs

                grows = [None] * 4  # (g1m_row fp16, g_row fp16) per i-block

                def emit_gate(ib):
                    # z = Qw @ Wv (dup pair cols so both outputs land on
                    # partition 0); 1-g = sigmoid(-z-bv), g = sigmoid(z+bv)
                    pzc = zpool.tile([2, 512], FP32, tag="zr", name="zr")
                    for a in range(4):
                        nc.tensor.matmul(
                            pzc[:], misc_r[:, 8 + 2 * a:10 + 2 * a],
                            qwt[a][:, ib * 512:(ib + 1) * 512],
                            start=(a == 0), stop=(a == 3))
                    g1 = gtmp.tile([1, 512], FP16, tag="g1", name="g1")
                    nc.scalar.activation(g1[:], pzc[0:1, :], AF.Sigmoid,
                                         scale=-1.0, bias=misc_sb[0:1, 17:18])
                    g2 = gtmp.tile([1, 512], FP16, tag="g2", name="g2")
                    nc.scalar.activation(g2[:], pzc[0:1, :], AF.Sigmoid,
                                         bias=misc_sb[0:1, 16:17])
                    grows[ib] = (g1, g2)

                def emit_bcast(ib):
                    # local planes for i-block ib via rank-1 ones matmuls
                    g1, g2 = grows[ib]
                    sl = slice(ib * 512, (ib + 1) * 512)
                    pb = bps.tile([128, 512], FP32, tag="pb", name="pb")
                    nc.tensor.matmul(pb[:], ones_h[:], g1[:],
                                     start=True, stop=True)
                    nc.vector.tensor_copy(g1m_bc[:, sl], pb[:])
                    pb2 = bps.tile([128, 512], FP32, tag="pb", name="pb")
                    nc.tensor.matmul(pb2[:], ones_h[:], g2[:],
                                     start=True, stop=True)
                    nc.scalar.copy(g_bc[:, sl], pb2[:])

                def emit_exchange(half):
                    # ship qwt[:, half*1024:(half+1)*1024] + the two gate rows
                    # for blocks 2*half, 2*half+1 through the pair AllGather
                    sl = slice(half * 1024, (half + 1) * 1024)
                    for a in range(4):
                        xc = xch.tile([128, 1024], FP16, tag="xc", name="xc")
                        nc.vector.tensor_copy(xc[:], qwt[a][:, sl])
                        nc.gpsimd.dma_start(agin[half][a * 128:(a + 1) * 128, :],
                                            xc[:])
                    for k, row in ((0, 512), (1, 513)):
                        for b2 in range(2):
                            ib = 2 * half + b2
                            nc.gpsimd.dma_start(
                                agin[half][row:row + 1,
                                           b2 * 512:(b2 + 1) * 512],
                                grows[ib][k][:])
                    nc.gpsimd.collective_compute(
                        "AllGather", ALU.bypass,
                        replica_groups=[[0, 1], [2, 3], [4, 5], [6, 7]],
                        ins=[agin[half][:].opt()], outs=[agout[half][:].opt()],
                    )

                # ---- Q pass (local i-blocks), exchange fired as halves
                # complete; gate deferred by one block so its z-matmuls never
                # head-block ready projection matmuls
                xs_next = load_x(0)
                for ib in range(4):
                    xs = xs_next
                    if ib == 0:
                        xs_next = load_x(1)
                        for h in range(8):
                            load_w(wk, h, wkr, "wkr")
                    else:
                        xs_next = load_x(ib + 1) if ib < 3 else None
                    for a in range(4):
                        pq = ppool.tile([128, 512], FP32, tag="ps", name="ps")
                        for h in range(8):
                            nc.tensor.matmul(pq[:], wqr[h][:, a * 128:(a + 1) * 128],
                                             xs[h][:], start=(h == 0), stop=(h == 7))
                        nc.scalar.activation(qwt[a][:, ib * 512:(ib + 1) * 512],
                                             pq[:], AF.Identity,
                                             bias=misc_sb[:, a:a + 1])
                    if ib >= 1:
                        emit_gate(ib - 1)
                    if ib == 2:
                        emit_exchange(0)

                # ---- K pass (x re-loaded from HBM; the sync queue is quiet
                # here and re-staging is cheaper than keeping 32KB/partition
                # of x tiles alive)
                xs_next = load_x(0)
                for ib in range(4):
                    xs = xs_next
                    xs_next = load_x(ib + 1) if ib < 3 else None
                    for a in range(4):
                        pk = ppool.tile([128, 512], FP32, tag="ps", name="ps")
                        for h in range(8):
                            nc.tensor.matmul(pk[:], wkr[h][:, a * 128:(a + 1) * 128],
                                             xs[h][:], start=(h == 0), stop=(h == 7))
                        nc.scalar.activation(kwt[a][:, ib * 512:(ib + 1) * 512],
                                             pk[:], AF.Identity,
                                             bias=misc_sb[:, 4 + a:5 + a])
                    if ib == 0:
                        emit_gate(3)
                        emit_exchange(1)
                    if ib >= 1:
                        emit_bcast(ib - 1)
                emit_bcast(3)

            # ---- exchange readback (indirect gathers run on gpsimd and
            # wait on the collective's completion; emitted in program order
            # AFTER the prefix score tiles so the waits never head-block
            # ready PE work)
            def readback(half):
                sl = slice(NSH + half * 1024, NSH + (half + 1) * 1024)
                for a in range(4):
                    gt = xch.tile([128, 1024], FP16, tag="gt", name="gt")
                    nc.gpsimd.indirect_dma_start(
                        out=gt[:], out_offset=None,
                        in_=agout[half][:],
                        in_offset=bass.IndirectOffsetOnAxis(
                            ap=idx_sb[:, a:a + 1], axis=0),
                    )
                    nc.vector.tensor_copy(qwt[a][:, sl], gt[:])
                grem = grp.tile([2, 1024], FP16, tag="gr", name="gr")
                nc.gpsimd.indirect_dma_start(
                    out=grem[:], out_offset=None,
                    in_=agout[half][:],
                    in_offset=bass.IndirectOffsetOnAxis(
                        ap=idx_sb[0:2, 4:5], axis=0),
                )
                return grem

            def remote_bcast(half, grem, bps_pool):
                # remote g1m plane for the two 512-blocks of this half
                for b2 in range(2):
                    sl = slice(NSH + half * 1024 + b2 * 512,
                               NSH + half * 1024 + (b2 + 1) * 512)
                    pb = bps_pool.tile([128, 512], FP32, tag="pb", name="pb")
                    nc.tensor.matmul(pb[:], ones_h[:],
                                     grem[0:1, b2 * 512:(b2 + 1) * 512],
                                     start=True, stop=True)
                    nc.vector.tensor_copy(g1m_bc[:, sl], pb[:])

            # ---- score loop over 16 column tiles (output stays transposed)
            with (
                tc.tile_pool(name="explo", bufs=11) as eplo,
                tc.tile_pool(name="exphi", bufs=3) as ephi,
                tc.tile_pool(name="dsum", bufs=20) as dpool,
                tc.tile_pool(name="diag", bufs=2) as dzpool,
                tc.tile_pool(name="scoreps", bufs=3, space="PSUM") as sps,
                tc.tile_pool(name="bcps2", bufs=2, space="PSUM") as bps2,
            ):
                exp_lo = [None] * 16
                dsums = [None] * 16

                def score_half(t, lo):
                    # compute the two 1024-chunks of one half of tile t
                    if lo:
                        et = eplo.tile([128, NSH], BF16, tag="el", name="el")
                        exp_lo[t] = et
                        ds = dpool.tile([128, 4], FP32, tag="ds", name="ds")
                        dsums[t] = ds
                        chs = (0, 1)
                    else:
                        et = ephi.tile([128, NSH], BF16, tag="eh", name="eh")
                        ds = dsums[t]
                        chs = (2, 3)
                    dch = (t * 128) // 1024
                    for ch in chs:
                        ps = sps.tile([128, 1024], FP32, tag="sc", name="sc")
                        for sub in range(2):
                            o = ch * 1024 + sub * 512
                            for a in range(4):
                                nc.tensor.matmul(ps[:, sub * 512:(sub + 1) * 512],
                                                 kwt[a][:, t * 128:(t + 1) * 128],
                                                 qwt[a][:, o:o + 512],
                                                 start=(a == 0), stop=(a == 3))
                        if ch == dch:
                            off = t * 128 - ch * 1024
                            nc.vector.tensor_add(ps[:, off:off + 128],
                                                 ps[:, off:off + 128], dneg[:])
                        co = (ch - chs[0]) * 1024
                        nc.scalar.activation(et[:, co:co + 1024], ps[:],
                                             AF.Exp, accum_out=ds[:, ch:ch + 1])
                    return et

                def finalize(t, ehi, last=False):
                    elo = exp_lo[t]
                    ds = dsums[t]
                    rcol = dpool.tile([128, 1], FP32, tag="r", name="r")
                    nc.vector.tensor_reduce(rcol[:], ds[:], axis=AX.X,
                                            op=ALU.add)
                    nc.vector.reciprocal(rcol[:], rcol[:])
                    eng, eng2 = ((nc.sync, nc.gpsimd) if t % 2 == 1
                                 else (nc.gpsimd, nc.sync))
                    for ch in range(2):
                        sl = slice(ch * 1024, (ch + 1) * 1024)
                        nc.vector.scalar_tensor_tensor(
                            elo[:, sl], elo[:, sl], rcol[:],
                            g1m_bc[:, sl], op0=ALU.mult, op1=ALU.mult)
                    dz = dzpool.tile([128, 128], BF16, tag="dz", name="dz")
                    nc.vector.tensor_mul(dz[:], identb[:],
                                         g_bc[:, t * 128:(t + 1) * 128])
                    nc.vector.tensor_add(elo[:, t * 128:(t + 1) * 128],
                                         elo[:, t * 128:(t + 1) * 128], dz[:])
                    eng.dma_start(out[t * 128:(t + 1) * 128, 0:NSH], elo[:])
                    for ch in range(2):
                        sl = slice(ch * 1024, (ch + 1) * 1024)
                        nc.vector.scalar_tensor_tensor(
                            ehi[:, sl], ehi[:, sl], rcol[:],
                            g1m_bc[:, NSH + ch * 1024:NSH + (ch + 1) * 1024],
                            op0=ALU.mult, op1=ALU.mult)
                        if last:
                            e = eng if ch == 0 else eng2
                            e.dma_start(
                                out[t * 128:(t + 1) * 128,
                                    NSH + ch * 1024:NSH + (ch + 1) * 1024],
                                ehi[:, sl])
                    if not last:
                        eng2.dma_start(out[t * 128:(t + 1) * 128, NSH:N],
                                       ehi[:])

                # prefix: local-column halves only (latency buffer for the AG)
                for t in range(PREFIX):
                    score_half(t, lo=True)
                # remote half readback + plane broadcast
                grem0 = readback(0)
                remote_bcast(0, grem0, bps2)
                grem1 = readback(1)
                remote_bcast(1, grem1, bps2)
                # drain prefix tiles, then the rest fully
                for t in range(16):
                    if t >= PREFIX:
                        score_half(t, lo=True)
                    ehi = score_half(t, lo=False)
                    finalize(t, ehi, last=(t == 15))
    nc.compile()
    return nc


def kernel(x, Wq, bq, Wk, bk, Wv, bv, _trace=False, _tmpdir=None):
    x = np.asarray(x, dtype=np.float32)
    if "nc" not in _CACHE:
        _CACHE["nc"] = _build()
    nc = _CACHE["nc"]

    bv_f = np.float32(np.asarray(bv).reshape(())[()])
    eye_np = np.eye(128, dtype=np.float32)
    misc = np.zeros((128, 18), dtype=np.float32)
    misc[:, 0:4] = np.asarray(bq, np.float32).reshape(4, 128).T
    misc[:, 4:8] = np.asarray(bk, np.float32).reshape(4, 128).T
    wv_c = np.asarray(Wv, np.float32).reshape(4, 128).T
    misc[:, 8:16:2] = wv_c
    misc[:, 9:16:2] = wv_c
    misc[:, 16] = bv_f
    misc[:, 17] = -bv_f
    wq_np = np.ascontiguousarray(np.asarray(Wq, np.float32).astype(np.float16))
    wk_np = np.ascontiguousarray(np.asarray(Wk, np.float32).astype(np.float16))

    in_maps = []
    for c in range(8):
        b, h = c // 2, c % 2
        # local rows only: perm-i [0,2048) = orig rows [h*2048,(h+1)*2048)
        xqc = np.ascontiguousarray(
            x[b].T[:, h * NSH:(h + 1) * NSH].astype(np.float16))
        idx = np.zeros((128, 5), dtype=np.int32)
        base = (1 - h) * AGR
        for a in range(4):
            idx[:, a] = base + a * 128 + np.arange(128)
        idx[0:2, 4] = base + 512 + np.arange(2)
        in_maps.append({"xq": xqc, "wq": wq_np, "wk": wk_np, "misc": misc,
                        "eye": eye_np, "idx": idx})

    res = run_bass_kernel_spmd(nc, in_maps, list(range(8)), trace=_trace,
                               tmpdir=_tmpdir)

    outp = np.empty((B, N, N), dtype=np.float32)
    for c in range(8):
        b, h = c // 2, c % 2
        O = np.asarray(res.results[c]["out"]).astype(np.float32).T  # (i_perm, j)
        js = slice(h * NSH, (h + 1) * NSH)
        outp[b, h * NSH:(h + 1) * NSH, js] = O[:NSH]
        outp[b, (1 - h) * NSH:(2 - h) * NSH, js] = O[NSH:]
    if _trace:
        return outp, res
    return outp
